# revision 8
# baseline (speedup 1.0000x reference)
"""GCN classifier (2-layer mean-agg GCN + mean pooling + linear head) on 8 TRN2 cores.

Strategy (v2, slot-based aggregation):
- Partition dst nodes across 8 cores (12500 each). Window = 128 dst nodes
  (lanes). Per (window, chunk) a slot rectangle [128 lanes x S slots] holds
  each lane's edges at ranks 0..deg-1; padding cells gather a zero row.
- Gather: gpsimd.dma_gather with prepare_only=True + trigger_dma on 4 queues,
  so descriptor generation overlaps DMA and compute.
- Aggregation per window: DVE tensor_reduce over the slot axis (one partial
  per chunk, one combine reduce), then scale by 1/deg, PE transpose, dense
  W matmul, relu+bias on ACT, PE transpose back to node-major.
- h1 exchange: 4 quarter-AllGathers (3125 rows each) pipelined against the
  L1 window loop; each chunk table carries its own zero row for padding.
- Pooling: host-precomputed per-window graph one-hots (scaled by 1/cnt)
  matmul-accumulated into one PSUM bank inside the L2 window loop;
  AllReduce + linear head at the end.
"""

import os

import numpy as np

N = 100_000
E = 1_600_000
D = 64
V = 50_000
G = 128
C = 20
NCORES = 8
S_ = N // NCORES          # 12500 nodes per core
W = 128                   # window width (dst nodes per window)
NW = (S_ + W - 1) // W    # 98 windows per core
WG = 4                    # windows per gather group
NG = (NW + WG - 1) // WG  # 25 groups
QW = S_ // 4              # 3125 rows per quarter-AllGather
NCH1 = 2                  # emb chunks (25000 rows each, int16 idx limit)
CW1 = V // NCH1           # 25000
NCH2 = 4                  # h1 chunks (one per quarter-AllGather)
CHROWS1 = CW1 + 1         # chunk rows incl zero row
CHROWS2 = 8 * QW + 1      # 25001

f32 = np.float32

last_result = None  # results of the most recent run (for test.py)


def _slot_struct(core, w_all, ch_all, lane_all, nch):
    """Compile-time slot structure shared by all cores: S[w, k] = max edge
    count over (cores, lanes); group/call/window offset tables."""
    key = ((core * NW + w_all) * nch + ch_all) * W + lane_all
    cnt = np.bincount(key, minlength=NCORES * NW * nch * W)
    cnt = cnt.reshape(NCORES, NW, nch, W)
    Smat = cnt.max(axis=(0, 3)).astype(np.int64)  # [NW, nch]

    off_wk = np.zeros((NW, nch), np.int64)  # absolute slot offset of (w, k)
    groups = []
    pos = 0
    for g in range(NG):
        ws = list(range(g * WG, min((g + 1) * WG, NW)))
        slot0 = pos
        calls = []  # (k, Tgk, rel_off_slots)
        for k in range(nch):
            rel = pos - slot0
            Tgk = 0
            for w in ws:
                off_wk[w, k] = pos
                pos += int(Smat[w, k])
                Tgk += int(Smat[w, k])
            if Tgk > 0:
                calls.append((k, Tgk, rel))
        windows = []
        for w in ws:
            runs = [
                (k, int(off_wk[w, k] - slot0), int(Smat[w, k]))
                for k in range(nch)
            ]
            windows.append((w, runs))
        groups.append(dict(slot0=slot0, Tg=pos - slot0, calls=calls, windows=windows))
    return dict(S=Smat, T=pos, groups=groups, nch=nch)


def _slot_idx(struct, mask, w_all, ch_all, lane_all, li_all, zr):
    """Per-core idx stream for one layer: [128, T*8] int16 wrapped format."""
    nch = struct["nch"]
    T = struct["T"]
    # absolute slot offset lookup rebuilt from groups
    off_wk = np.zeros((NW, nch), np.int64)
    for grp in struct["groups"]:
        for w, runs in grp["windows"]:
            for k, rel, Swk in runs:
                off_wk[w, k] = grp["slot0"] + rel
    w_m = w_all[mask]
    k_m = ch_all[mask]
    lane_m = lane_all[mask]
    li_m = li_all[mask]
    okey = (w_m * nch + k_m) * W + lane_m
    order = np.argsort(okey, kind="stable")
    ok_s = okey[order]
    li_s = li_m[order]
    run_start = np.searchsorted(ok_s, ok_s, side="left")
    rank = np.arange(len(ok_s)) - run_start
    w_s = ok_s // (nch * W)
    k_s = (ok_s // W) % nch
    lane_s = ok_s % W
    pos = (off_wk[w_s, k_s] + rank) * W + lane_s
    flat = np.full(T * W, zr, np.int16)
    flat[pos] = li_s.astype(np.int16)
    return np.tile(np.ascontiguousarray(flat.reshape(T * 8, 16).T), (8, 1))


def _prep(tokens, edge_src, edge_dst, graph_ids):
    deg = np.bincount(edge_dst, minlength=N).astype(np.int64)
    invdeg = (1.0 / np.maximum(deg, 1.0)).astype(f32)

    core = edge_dst // S_
    dloc = edge_dst - core * S_
    w_all = dloc // W
    lane_all = dloc - w_all * W

    # L1: gather emb row tokens[src]; chunk by token//25000
    t1 = tokens[edge_src]
    ch1 = t1 // CW1
    li1 = t1 - ch1 * CW1
    # L2: gather h1 row by src position in the quarter-AllGather layout:
    # h1g[q] row 3125*c + j  <->  global node 12500*c + 3125*q + j
    sc = edge_src // S_
    sloc = edge_src - sc * S_
    q2 = sloc // QW
    li2 = QW * sc + (sloc - q2 * QW)

    s1 = _slot_struct(core, w_all, ch1, lane_all, NCH1)
    s2 = _slot_struct(core, w_all, q2, lane_all, NCH2)

    idx1, idx2, invd, pooloh = [], [], [], []
    cnt = np.bincount(graph_ids, minlength=G).astype(f32)
    invcnt = (1.0 / np.maximum(cnt, 1.0)).astype(f32)
    for c in range(NCORES):
        m = core == c
        idx1.append(_slot_idx(s1, m, w_all, ch1, lane_all, li1, CW1))
        idx2.append(_slot_idx(s2, m, w_all, q2, lane_all, li2, 8 * QW))
        iv = np.zeros(NW * W, f32)
        iv[:S_] = invdeg[c * S_ : (c + 1) * S_]
        invd.append(np.ascontiguousarray(iv.reshape(NW, W).T))
        oh = np.zeros((NW * W, G), f32)
        gids = graph_ids[c * S_ : (c + 1) * S_]
        oh[np.arange(S_), gids] = invcnt[gids]
        # [128 lanes, NW*G]: window w cols [w*G, (w+1)*G)
        pooloh.append(
            np.ascontiguousarray(
                oh.reshape(NW, W, G).transpose(1, 0, 2).reshape(W, NW * G)
            )
        )
    return s1, s2, idx1, idx2, invd, pooloh, cnt


def _build(s1, s2):
    import concourse.bacc as bacc
    import concourse.mybir as mybir
    import concourse.tile as tile

    dt = mybir.dt
    Alu = mybir.AluOpType
    Act = mybir.ActivationFunctionType
    Ax = mybir.AxisListType

    nq = int(os.environ.get("GNN_NQ", "4"))
    nc = bacc.Bacc(
        "TRN2",
        target_bir_lowering=False,
        debug=False,
        num_devices=NCORES,
        num_swdge_queues=nq,
    )

    embg = nc.dram_tensor("embg", [NCH1 * CHROWS1, D], dt.float32, kind="ExternalInput")
    consts = nc.dram_tensor("consts", [128, 128], dt.float32, kind="ExternalInput")
    wts = nc.dram_tensor("wts", [64, 130], dt.float32, kind="ExternalInput")
    head = nc.dram_tensor("head", [1, 128 + C], dt.float32, kind="ExternalInput")
    wc = nc.dram_tensor("wc", [64, C], dt.float32, kind="ExternalInput")
    invd = nc.dram_tensor("invd", [128, NW], dt.float32, kind="ExternalInput")
    pooloh = nc.dram_tensor("pooloh", [128, NW * G], dt.float32, kind="ExternalInput")
    l1idx = nc.dram_tensor("l1idx", [128, s1["T"] * 8], dt.int16, kind="ExternalInput")
    l2idx = nc.dram_tensor("l2idx", [128, s2["T"] * 8], dt.int16, kind="ExternalInput")
    logits = nc.dram_tensor("logits", [G, C], dt.float32, kind="ExternalOutput")
    dbg_h1 = os.environ.get("GNN_DBG_H1", "0") == "1"
    h1dbg = (
        nc.dram_tensor("h1dbg", [S_, D], dt.float32, kind="ExternalOutput")
        if dbg_h1
        else None
    )

    h1_shard = nc.dram_tensor("h1_shard", [S_, D], dt.float32, kind="Internal")
    h1g = [
        nc.dram_tensor(f"h1g{q}", [CHROWS2, D], dt.float32, kind="Internal",
                       addr_space="Shared")
        for q in range(4)
    ]
    pooled_in = nc.dram_tensor("pooled_in", [64, G], dt.float32, kind="Internal")
    pooled_out = nc.dram_tensor(
        "pooled_out", [64, G], dt.float32, kind="Internal", addr_space="Shared"
    )

    dsems = [nc.alloc_semaphore(f"gdma{q}") for q in range(nq)]

    with tile.TileContext(nc, num_cores=NCORES) as tc:
        with (
            tc.tile_pool(name="const", bufs=1) as cpool,
            tc.tile_pool(name="gsl", bufs=int(os.environ.get("GNN_GBUFS", "3"))) as gpool,
            tc.tile_pool(name="md", bufs=3) as mpool,
            tc.tile_pool(name="pt", bufs=3) as ppool,
            tc.tile_pool(name="act", bufs=4) as apool,
            tc.tile_pool(name="ps", bufs=2, space="PSUM") as pspool,
            tc.tile_pool(name="pg", bufs=1, space="PSUM") as pgpool,
        ):
            consts_t = cpool.tile([128, 128], dt.float32)
            nc.sync.dma_start(consts_t[:], consts[:])
            ident = consts_t[:, 0:128]
            wts_t = cpool.tile([64, 130], dt.float32)
            nc.sync.dma_start(wts_t[:], wts[:])
            head_t = cpool.tile([1, 128 + C], dt.float32)
            nc.sync.dma_start(head_t[:], head[:])
            wc_t = cpool.tile([64, C], dt.float32)
            nc.sync.dma_start(wc_t[:], wc[:])
            invd_t = cpool.tile([128, NW], dt.float32)
            nc.sync.dma_start(invd_t[:], invd[:])

            # zero rows of the h1 gather chunks
            zrow = cpool.tile([1, D], dt.float32)
            nc.vector.memset(zrow[:], 0.0)
            for q in range(4):
                nc.sync.dma_start(h1g[q][8 * QW : 8 * QW + 1, :], zrow[:])

            pool_ps = pgpool.tile([64, G], dt.float32)

            for L, (st, idx_d) in enumerate([(s1, l1idx), (s2, l2idx)]):
                nch = st["nch"]
                if L == 0:
                    views = [
                        embg[ch * CHROWS1 : (ch + 1) * CHROWS1, :]
                        for ch in range(NCH1)
                    ]
                else:
                    views = [h1g[q][:] for q in range(4)]
                Wl = wts_t[:, L * 64 : (L + 1) * 64]
                bl = wts_t[:, 128 + L : 129 + L]

                for gi, grp in enumerate(st["groups"]):
                    Tg = grp["Tg"]
                    slot0 = grp["slot0"]
                    idx_sl = mpool.tile([128, Tg * 8], dt.int16, tag="idx")
                    nc.sync.dma_start(idx_sl[:], idx_d[:, slot0 * 8 : (slot0 + Tg) * 8])
                    slab = gpool.tile([128, Tg * 64], dt.float32, tag="slab")
                    use_prep = os.environ.get("GNN_PREP", "1") == "1"
                    used_q = set()
                    for k, Tgk, rel in grp["calls"]:
                        q = k % nq
                        used_q.add(q)
                        kw = (
                            dict(prepare_only=True, sem=dsems[q])
                            if use_prep
                            else {}
                        )
                        nc.gpsimd.dma_gather(
                            out_ap=slab[:, rel * 64 : (rel + Tgk) * 64].rearrange(
                                "p (t d) -> p t d", d=64
                            ),
                            in_ap=views[k],
                            idxs_ap=idx_sl[:, rel * 8 : (rel + Tgk) * 8],
                            num_idxs=Tgk * 128,
                            num_idxs_reg=Tgk * 128,
                            elem_size=64,
                            single_packet=False,
                            queue_num=q,
                            **kw,
                        )
                    if use_prep:
                        for q in sorted(used_q):
                            nc.gpsimd.trigger_dma(count=None, queue_num=q)

                    if L == 1:
                        posl = mpool.tile([128, WG * G], dt.float32, tag="poh")
                        w0 = grp["windows"][0][0]
                        nwin = len(grp["windows"])
                        nc.sync.dma_start(
                            posl[:, : nwin * G],
                            pooloh[:, w0 * G : (w0 + nwin) * G],
                        )

                    for w, runs in grp["windows"]:
                        live = [(k, rel, Swk) for k, rel, Swk in runs if Swk > 0]
                        if len(live) == 0:
                            aggs = apool.tile([128, 64], dt.float32, tag="aggs")
                            nc.vector.memset(aggs[:], 0.0)
                        else:
                            if len(live) == 1:
                                k, rel, Swk = live[0]
                                agg_in = apool.tile([128, 64], dt.float32, tag="part")
                                nc.vector.tensor_reduce(
                                    agg_in[:],
                                    slab[:, rel * 64 : (rel + Swk) * 64].rearrange(
                                        "p (s d) -> p d s", d=64
                                    ),
                                    Ax.X,
                                    Alu.add,
                                )
                            else:
                                parts = ppool.tile(
                                    [128, len(live) * 64], dt.float32, tag="part"
                                )
                                for j, (k, rel, Swk) in enumerate(live):
                                    nc.vector.tensor_reduce(
                                        parts[:, j * 64 : (j + 1) * 64],
                                        slab[
                                            :, rel * 64 : (rel + Swk) * 64
                                        ].rearrange("p (s d) -> p d s", d=64),
                                        Ax.X,
                                        Alu.add,
                                    )
                                agg_in = apool.tile([128, 64], dt.float32, tag="agg")
                                nc.vector.tensor_reduce(
                                    agg_in[:],
                                    parts[:].rearrange("p (k d) -> p d k", d=64),
                                    Ax.X,
                                    Alu.add,
                                )
                            aggs = apool.tile([128, 64], dt.float32, tag="aggs")
                            nc.vector.tensor_scalar(
                                aggs[:], agg_in[:], invd_t[:, w : w + 1], None, Alu.mult
                            )
                        t1p = pspool.tile([64, 128], dt.float32, tag="t1")
                        nc.tensor.transpose(t1p[:], aggs[:], ident)
                        aggT = apool.tile([64, 128], dt.float32, tag="aggT")
                        nc.scalar.copy(aggT[:], t1p[:])
                        zps = pspool.tile([64, 128], dt.float32, tag="z")
                        nc.tensor.matmul(zps[:], lhsT=Wl, rhs=aggT[:], start=True, stop=True)
                        hT = apool.tile([64, 128], dt.float32, tag="hT")
                        nc.scalar.activation(hT[:], zps[:], Act.Relu, bias=bl)
                        t2p = pspool.tile([128, 64], dt.float32, tag="t2")
                        nc.tensor.transpose(t2p[:], hT[:], ident[:64, :64])
                        hwv = apool.tile([128, 64], dt.float32, tag="hw")
                        nc.scalar.copy(hwv[:], t2p[:])
                        if L == 0:
                            rows = min(128, S_ - w * 128)
                            nc.sync.dma_start(
                                h1_shard[w * 128 : w * 128 + rows, :], hwv[:rows, :]
                            )
                            if dbg_h1:
                                nc.sync.dma_start(
                                    h1dbg[w * 128 : w * 128 + rows, :], hwv[:rows, :]
                                )
                        else:
                            wrel = w - grp["windows"][0][0]
                            nc.tensor.matmul(
                                pool_ps[:],
                                lhsT=hwv[:],
                                rhs=posl[:, wrel * G : (wrel + 1) * G],
                                start=(w == 0),
                                stop=(w == NW - 1),
                            )

                    if L == 0:
                        for q in range(4):
                            if gi == [6, 12, 18, 24][q]:
                                nc.gpsimd.collective_compute(
                                    "AllGather",
                                    Alu.bypass,
                                    replica_groups=[list(range(NCORES))],
                                    ins=[h1_shard[q * QW : (q + 1) * QW, :]],
                                    outs=[h1g[q][0 : 8 * QW, :]],
                                )

            pooled_sb = apool.tile([64, G], dt.float32, tag="aggT")
            nc.scalar.copy(pooled_sb[:], pool_ps[:])
            nc.sync.dma_start(pooled_in[:], pooled_sb[:])
            nc.gpsimd.collective_compute(
                "AllReduce",
                Alu.add,
                replica_groups=[list(range(NCORES))],
                ins=[pooled_in[:]],
                outs=[pooled_out[:]],
            )
            pooledT = apool.tile([64, G], dt.float32, tag="hT")
            nc.sync.dma_start(pooledT[:], pooled_out[:])

            # head: logits = pooledT.T @ Wc + ones (x) bc
            ones_row = head_t[:, 0:128]
            bc_row = head_t[:, 128 : 128 + C]
            lps = pspool.tile([G, C], dt.float32, tag="z")
            nc.tensor.matmul(lps[:], lhsT=pooledT[:], rhs=wc_t[:], start=True, stop=False)
            nc.tensor.matmul(lps[:], lhsT=ones_row, rhs=bc_row, start=False, stop=True)
            lsb = apool.tile([G, C], dt.float32, tag="hw")
            nc.scalar.copy(lsb[:], lps[:])
            nc.sync.dma_start(logits[:], lsb[:])

    nc.finalize()
    return nc


def _run_timed(nc, in_maps, iters=1):
    """Mirror bass2jax.run_bass_via_pjrt's multi-core path, but keep inputs on
    device and execute `iters` times, timing each execution. Returns
    (results, times_s)."""
    import time

    import jax
    import numpy as _np
    from jax.experimental.shard_map import shard_map
    from jax.sharding import Mesh, NamedSharding, PartitionSpec

    import concourse.mybir as mybir
    from concourse import bass2jax

    bass2jax.install_neuronx_cc_hook()
    n_cores = len(in_maps)
    partition_name = nc.partition_id_tensor.name if nc.partition_id_tensor else None

    in_names, out_names, out_avals, zero_outs = [], [], [], []
    for alloc in nc.m.functions[0].allocations:
        if not isinstance(alloc, mybir.MemoryLocationSet):
            continue
        name = alloc.memorylocations[0].name
        if alloc.kind == "ExternalInput":
            if name != partition_name:
                in_names.append(name)
        elif alloc.kind == "ExternalOutput":
            out_names.append(name)
            shape = tuple(alloc.tensor_shape)
            dtype = mybir.dt.np(alloc.dtype)
            out_avals.append(jax.core.ShapedArray(shape, dtype))
            zero_outs.append(_np.zeros(shape, dtype))
    n_params = len(in_names)
    n_outs = len(out_avals)
    all_in_names = list(in_names) + out_names
    if partition_name is not None:
        all_in_names.append(partition_name)
    donate = tuple(range(n_params, n_params + n_outs))

    def _body(*args):
        operands = list(args)
        if partition_name is not None:
            operands.append(bass2jax.partition_id_tensor())
        outs = bass2jax._bass_exec_p.bind(
            *operands,
            out_avals=tuple(out_avals),
            in_names=tuple(all_in_names),
            out_names=tuple(out_names),
            lowering_input_output_aliases=(),
            sim_require_finite=True,
            sim_require_nnan=True,
            nc=nc,
        )
        return tuple(outs)

    devices = jax.devices()[:n_cores]
    mesh = Mesh(np.asarray(devices), ("core",))
    in_specs = (PartitionSpec("core"),) * (n_params + n_outs)
    out_specs = (PartitionSpec("core"),) * n_outs
    sharded = jax.jit(
        shard_map(_body, mesh=mesh, in_specs=in_specs, out_specs=out_specs, check_rep=False),
        donate_argnums=donate,
        keep_unused=True,
    )
    sh = NamedSharding(mesh, PartitionSpec("core"))
    concat_in = [
        jax.device_put(
            _np.concatenate([_np.asarray(in_maps[c][nm]) for c in range(n_cores)], axis=0),
            sh,
        )
        for nm in in_names
    ]
    times = []
    out_arrs = None
    for _ in range(max(1, iters)):
        concat_zeros = [
            jax.device_put(_np.zeros((n_cores * z.shape[0], *z.shape[1:]), z.dtype), sh)
            for z in zero_outs
        ]
        jax.block_until_ready(concat_zeros)
        t0 = time.perf_counter()
        out_arrs = sharded(*concat_in, *concat_zeros)
        jax.block_until_ready(out_arrs)
        times.append(time.perf_counter() - t0)
    B = int(os.environ.get("GNN_PIPE", "0"))
    if B > 1:
        zsets = [
            [
                jax.device_put(
                    _np.zeros((n_cores * z.shape[0], *z.shape[1:]), z.dtype), sh
                )
                for z in zero_outs
            ]
            for _ in range(B)
        ]
        jax.block_until_ready(zsets)
        t0 = time.perf_counter()
        outs = [sharded(*concat_in, *zs) for zs in zsets]
        jax.block_until_ready(outs)
        tot = time.perf_counter() - t0
        single = min(times)
        print(
            f"pipelined x{B}: total={tot * 1e3:.2f}ms  single={single * 1e3:.2f}ms  "
            f"marginal={(tot - single) / (B - 1) * 1e6:.0f}us"
        )
        times.append(max((tot - single) / (B - 1), 1e-9))
    results = [
        {
            nm: _np.asarray(out_arrs[i]).reshape(n_cores, *out_avals[i].shape)[c]
            for i, nm in enumerate(out_names)
        }
        for c in range(n_cores)
    ]
    return results, times


def build(inputs):
    """Build (nc, in_maps) for the current inputs. Shared by kernel() and
    the profiling harness."""
    tokens = np.asarray(inputs["tokens"]).astype(np.int64)
    edge_src = np.asarray(inputs["edge_src"]).astype(np.int64)
    edge_dst = np.asarray(inputs["edge_dst"]).astype(np.int64)
    graph_ids = np.asarray(inputs["graph_ids"]).astype(np.int64)
    emb = np.asarray(inputs["emb_table"], f32)
    W1 = np.asarray(inputs["W1"], f32)
    b1 = np.asarray(inputs["b1"], f32)
    W2 = np.asarray(inputs["W2"], f32)
    b2 = np.asarray(inputs["b2"], f32)
    Wc = np.asarray(inputs["Wc"], f32)
    bc = np.asarray(inputs["bc"], f32)

    s1, s2, idx1, idx2, invd, pooloh, cnt = _prep(tokens, edge_src, edge_dst, graph_ids)
    if os.environ.get("GNN_VERBOSE", "0") == "1":
        print(f"T1={s1['T']} tiles ({s1['T'] * 128} rows), "
              f"T2={s2['T']} tiles ({s2['T'] * 128} rows)")

    embg = np.zeros((NCH1 * CHROWS1, D), f32)
    for ch in range(NCH1):
        embg[ch * CHROWS1 : ch * CHROWS1 + CW1] = emb[ch * CW1 : (ch + 1) * CW1]

    consts = np.eye(128, dtype=f32)
    wts = np.concatenate([W1, W2, b1[:, None], b2[:, None]], axis=1)
    head_arr = np.zeros((1, 128 + C), f32)
    head_arr[0, :128] = 1.0
    head_arr[0, 128 : 128 + C] = bc

    nc = _build(s1, s2)

    in_maps = []
    for c in range(NCORES):
        in_maps.append(
            {
                "embg": embg,
                "consts": consts,
                "wts": wts,
                "head": head_arr,
                "wc": Wc,
                "invd": invd[c],
                "pooloh": pooloh[c],
                "l1idx": idx1[c],
                "l2idx": idx2[c],
            }
        )
    return nc, in_maps


def kernel(**inputs):
    global last_result

    nc, in_maps = build(inputs)

    iters = int(os.environ.get("GNN_BENCH", "1"))
    results, times = _run_timed(nc, in_maps, iters=iters)
    last_result = {"times": times, "results": results}
    if iters > 1:
        print(f"exec times (s): {[f'{t * 1e3:.2f}ms' for t in times]}")
        print(f"best exec: {min(times) * 1e6:.0f} us")
    return np.asarray(results[0]["logits"], f32)


# revision 10
# speedup vs baseline: 1.2059x; 1.2059x over previous
"""GCN classifier (2-layer mean-agg GCN + mean pooling + linear head) on 8 TRN2 cores.

Strategy (v2, slot-based aggregation):
- Partition dst nodes across 8 cores (12500 each). Window = 128 dst nodes
  (lanes). Per (window, chunk) a slot rectangle [128 lanes x S slots] holds
  each lane's edges at ranks 0..deg-1; padding cells gather a zero row.
- Gather: gpsimd.dma_gather with prepare_only=True + trigger_dma on 4 queues,
  so descriptor generation overlaps DMA and compute.
- Aggregation per window: DVE tensor_reduce over the slot axis (one partial
  per chunk, one combine reduce), then scale by 1/deg, PE transpose, dense
  W matmul, relu+bias on ACT, PE transpose back to node-major.
- h1 exchange: 4 quarter-AllGathers (3125 rows each) pipelined against the
  L1 window loop; each chunk table carries its own zero row for padding.
- Pooling: host-precomputed per-window graph one-hots (scaled by 1/cnt)
  matmul-accumulated into one PSUM bank inside the L2 window loop;
  AllReduce + linear head at the end.
"""

import os

import numpy as np

N = 100_000
E = 1_600_000
D = 64
V = 50_000
G = 128
C = 20
NCORES = 8
S_ = N // NCORES          # 12500 nodes per core
W = 128                   # window width (dst nodes per window)
NW = (S_ + W - 1) // W    # 98 windows per core
WG = 4                    # windows per gather group
NG = (NW + WG - 1) // WG  # 25 groups
QW = S_ // 4              # 3125 rows per quarter-AllGather
NCH1 = 2                  # emb chunks (25000 rows each, int16 idx limit)
CW1 = V // NCH1           # 25000
NCH2 = 4                  # h1 chunks (one per quarter-AllGather)
CHROWS1 = CW1 + 1         # chunk rows incl zero row
CHROWS2 = 8 * QW + 1      # 25001

f32 = np.float32

last_result = None  # results of the most recent run (for test.py)


def _slot_struct(core, w_all, ch_all, lane_all, nch):
    """Compile-time slot structure shared by all cores: S[w, k] = max edge
    count over (cores, lanes); group/call/window offset tables."""
    key = ((core * NW + w_all) * nch + ch_all) * W + lane_all
    cnt = np.bincount(key, minlength=NCORES * NW * nch * W)
    cnt = cnt.reshape(NCORES, NW, nch, W)
    Smat = cnt.max(axis=(0, 3)).astype(np.int64)  # [NW, nch]

    off_wk = np.zeros((NW, nch), np.int64)  # absolute slot offset of (w, k)
    groups = []
    pos = 0
    for g in range(NG):
        ws = list(range(g * WG, min((g + 1) * WG, NW)))
        slot0 = pos
        calls = []  # (k, Tgk, rel_off_slots)
        for k in range(nch):
            rel = pos - slot0
            Tgk = 0
            for w in ws:
                off_wk[w, k] = pos
                pos += int(Smat[w, k])
                Tgk += int(Smat[w, k])
            if Tgk > 0:
                calls.append((k, Tgk, rel))
        windows = []
        for w in ws:
            runs = [
                (k, int(off_wk[w, k] - slot0), int(Smat[w, k]))
                for k in range(nch)
            ]
            windows.append((w, runs))
        groups.append(dict(slot0=slot0, Tg=pos - slot0, calls=calls, windows=windows))
    return dict(S=Smat, T=pos, groups=groups, nch=nch)


def _slot_idx(struct, mask, w_all, ch_all, lane_all, li_all, zr):
    """Per-core idx stream for one layer: [128, T*8] int16 wrapped format."""
    nch = struct["nch"]
    T = struct["T"]
    # absolute slot offset lookup rebuilt from groups
    off_wk = np.zeros((NW, nch), np.int64)
    for grp in struct["groups"]:
        for w, runs in grp["windows"]:
            for k, rel, Swk in runs:
                off_wk[w, k] = grp["slot0"] + rel
    w_m = w_all[mask]
    k_m = ch_all[mask]
    lane_m = lane_all[mask]
    li_m = li_all[mask]
    okey = (w_m * nch + k_m) * W + lane_m
    order = np.argsort(okey, kind="stable")
    ok_s = okey[order]
    li_s = li_m[order]
    run_start = np.searchsorted(ok_s, ok_s, side="left")
    rank = np.arange(len(ok_s)) - run_start
    w_s = ok_s // (nch * W)
    k_s = (ok_s // W) % nch
    lane_s = ok_s % W
    pos = (off_wk[w_s, k_s] + rank) * W + lane_s
    flat = np.full(T * W, zr, np.int16)
    flat[pos] = li_s.astype(np.int16)
    return np.tile(np.ascontiguousarray(flat.reshape(T * 8, 16).T), (8, 1))


def _prep(tokens, edge_src, edge_dst, graph_ids):
    deg = np.bincount(edge_dst, minlength=N).astype(np.int64)
    invdeg = (1.0 / np.maximum(deg, 1.0)).astype(f32)

    core = edge_dst // S_
    dloc = edge_dst - core * S_

    # L1: gather emb row tokens[src]; chunk by token//25000
    t1 = tokens[edge_src]
    ch1 = t1 // CW1
    li1 = t1 - ch1 * CW1

    # Per-core L1 window permutation: sort local nodes by (chunk0 edge count,
    # degree) so per-(window, chunk) maxima are tight -> minimal slot padding.
    w1 = np.zeros(E, np.int64)
    lane1 = np.zeros(E, np.int64)
    rowpos = np.zeros(N, np.int64)  # h1 shard row of each global node
    p1s = []
    for c in range(NCORES):
        m = core == c
        dl = dloc[m]
        cnt0 = np.bincount(dl[ch1[m] == 0], minlength=S_)
        dg = deg[c * S_ : (c + 1) * S_]
        p1 = np.lexsort((dg, cnt0))
        pos1 = np.empty(S_, np.int64)
        pos1[p1] = np.arange(S_)
        w1[m] = pos1[dl] // W
        lane1[m] = pos1[dl] - (pos1[dl] // W) * W
        rowpos[c * S_ : (c + 1) * S_] = pos1
        p1s.append(p1)

    # L2: gather h1 by shard row in the quarter-AllGather layout:
    # h1g[q] row 3125*c + j  <->  core c shard row 3125*q + j
    sc = edge_src // S_
    r2 = rowpos[edge_src]
    q2 = r2 // QW
    li2 = QW * sc + (r2 - q2 * QW)

    # Per-core L2 window permutation: sort by per-quarter counts then degree.
    w2 = np.zeros(E, np.int64)
    lane2 = np.zeros(E, np.int64)
    p2s = []
    for c in range(NCORES):
        m = core == c
        dl = dloc[m]
        cntq = np.bincount(dl * 4 + q2[m], minlength=S_ * 4).reshape(S_, 4)
        dg = cntq.sum(axis=1)
        p2 = np.lexsort((dg, cntq[:, 2], cntq[:, 1], cntq[:, 0]))
        pos2 = np.empty(S_, np.int64)
        pos2[p2] = np.arange(S_)
        w2[m] = pos2[dl] // W
        lane2[m] = pos2[dl] - (pos2[dl] // W) * W
        p2s.append(p2)

    s1 = _slot_struct(core, w1, ch1, lane1, NCH1)
    s2 = _slot_struct(core, w2, q2, lane2, NCH2)

    idx1, idx2, invd, pooloh = [], [], [], []
    cnt = np.bincount(graph_ids, minlength=G).astype(f32)
    invcnt = (1.0 / np.maximum(cnt, 1.0)).astype(f32)
    for c in range(NCORES):
        m = core == c
        idx1.append(_slot_idx(s1, m, w1, ch1, lane1, li1, CW1))
        idx2.append(_slot_idx(s2, m, w2, q2, lane2, li2, 8 * QW))
        # invd: [128, 2*NW]; cols L*NW + w, lane order per layer permutation
        iv = np.zeros((2, NW * W), f32)
        iv[0, :S_] = invdeg[c * S_ + p1s[c]]
        iv[1, :S_] = invdeg[c * S_ + p2s[c]]
        invd.append(
            np.ascontiguousarray(
                np.concatenate(
                    [iv[0].reshape(NW, W).T, iv[1].reshape(NW, W).T], axis=1
                )
            )
        )
        oh = np.zeros((NW * W, G), f32)
        gids = graph_ids[c * S_ + p2s[c]]
        oh[np.arange(S_), gids] = invcnt[gids]
        # [128 lanes, NW*G]: window w cols [w*G, (w+1)*G)
        pooloh.append(
            np.ascontiguousarray(
                oh.reshape(NW, W, G).transpose(1, 0, 2).reshape(W, NW * G)
            )
        )
    return s1, s2, idx1, idx2, invd, pooloh, cnt, p1s


def _build(s1, s2):
    import concourse.bacc as bacc
    import concourse.mybir as mybir
    import concourse.tile as tile

    dt = mybir.dt
    Alu = mybir.AluOpType
    Act = mybir.ActivationFunctionType
    Ax = mybir.AxisListType

    nq = int(os.environ.get("GNN_NQ", "4"))
    nc = bacc.Bacc(
        "TRN2",
        target_bir_lowering=False,
        debug=False,
        num_devices=NCORES,
        num_swdge_queues=nq,
    )

    embg = nc.dram_tensor("embg", [NCH1 * CHROWS1, D], dt.float32, kind="ExternalInput")
    consts = nc.dram_tensor("consts", [128, 128], dt.float32, kind="ExternalInput")
    wts = nc.dram_tensor("wts", [64, 130], dt.float32, kind="ExternalInput")
    head = nc.dram_tensor("head", [1, 128 + C], dt.float32, kind="ExternalInput")
    wc = nc.dram_tensor("wc", [64, C], dt.float32, kind="ExternalInput")
    invd = nc.dram_tensor("invd", [128, 2 * NW], dt.float32, kind="ExternalInput")
    pooloh = nc.dram_tensor("pooloh", [128, NW * G], dt.float32, kind="ExternalInput")
    l1idx = nc.dram_tensor("l1idx", [128, s1["T"] * 8], dt.int16, kind="ExternalInput")
    l2idx = nc.dram_tensor("l2idx", [128, s2["T"] * 8], dt.int16, kind="ExternalInput")
    logits = nc.dram_tensor("logits", [G, C], dt.float32, kind="ExternalOutput")
    dbg_h1 = os.environ.get("GNN_DBG_H1", "0") == "1"
    h1dbg = (
        nc.dram_tensor("h1dbg", [S_, D], dt.float32, kind="ExternalOutput")
        if dbg_h1
        else None
    )

    h1_shard = nc.dram_tensor("h1_shard", [S_, D], dt.float32, kind="Internal")
    h1g = [
        nc.dram_tensor(f"h1g{q}", [CHROWS2, D], dt.float32, kind="Internal",
                       addr_space="Shared")
        for q in range(4)
    ]
    pooled_in = nc.dram_tensor("pooled_in", [64, G], dt.float32, kind="Internal")
    pooled_out = nc.dram_tensor(
        "pooled_out", [64, G], dt.float32, kind="Internal", addr_space="Shared"
    )

    dsems = [nc.alloc_semaphore(f"gdma{q}") for q in range(nq)]

    with tile.TileContext(nc, num_cores=NCORES) as tc:
        with (
            tc.tile_pool(name="const", bufs=1) as cpool,
            tc.tile_pool(name="gsl", bufs=int(os.environ.get("GNN_GBUFS", "3"))) as gpool,
            tc.tile_pool(name="md", bufs=3) as mpool,
            tc.tile_pool(name="pt", bufs=3) as ppool,
            tc.tile_pool(name="act", bufs=4) as apool,
            tc.tile_pool(name="ps", bufs=2, space="PSUM") as pspool,
            tc.tile_pool(name="pg", bufs=1, space="PSUM") as pgpool,
        ):
            consts_t = cpool.tile([128, 128], dt.float32)
            nc.sync.dma_start(consts_t[:], consts[:])
            ident = consts_t[:, 0:128]
            wts_t = cpool.tile([64, 130], dt.float32)
            nc.sync.dma_start(wts_t[:], wts[:])
            head_t = cpool.tile([1, 128 + C], dt.float32)
            nc.sync.dma_start(head_t[:], head[:])
            wc_t = cpool.tile([64, C], dt.float32)
            nc.sync.dma_start(wc_t[:], wc[:])
            invd_t = cpool.tile([128, 2 * NW], dt.float32)
            nc.sync.dma_start(invd_t[:], invd[:])

            # zero rows of the h1 gather chunks
            zrow = cpool.tile([1, D], dt.float32)
            nc.vector.memset(zrow[:], 0.0)
            for q in range(4):
                nc.sync.dma_start(h1g[q][8 * QW : 8 * QW + 1, :], zrow[:])

            pool_ps = pgpool.tile([64, G], dt.float32)
            dma_count = [0] * nq

            for L, (st, idx_d) in enumerate([(s1, l1idx), (s2, l2idx)]):
                nch = st["nch"]
                if L == 0:
                    views = [
                        embg[ch * CHROWS1 : (ch + 1) * CHROWS1, :]
                        for ch in range(NCH1)
                    ]
                else:
                    views = [h1g[q][:] for q in range(4)]
                Wl = wts_t[:, L * 64 : (L + 1) * 64]
                bl = wts_t[:, 128 + L : 129 + L]

                for gi, grp in enumerate(st["groups"]):
                    Tg = grp["Tg"]
                    slot0 = grp["slot0"]
                    idx_sl = mpool.tile([128, Tg * 8], dt.int16, tag="idx")
                    nc.sync.dma_start(idx_sl[:], idx_d[:, slot0 * 8 : (slot0 + Tg) * 8])
                    slab = gpool.tile([128, Tg * 64], dt.float32, tag="slab")
                    use_prep = os.environ.get("GNN_PREP", "1") == "1"
                    used_q = set()
                    for k, Tgk, rel in grp["calls"]:
                        q = k % nq
                        used_q.add(q)
                        kw = (
                            dict(prepare_only=True, sem=dsems[q])
                            if use_prep
                            else {}
                        )
                        nc.gpsimd.dma_gather(
                            out_ap=slab[:, rel * 64 : (rel + Tgk) * 64].rearrange(
                                "p (t d) -> p t d", d=64
                            ),
                            in_ap=views[k],
                            idxs_ap=idx_sl[:, rel * 8 : (rel + Tgk) * 8],
                            num_idxs=Tgk * 128,
                            num_idxs_reg=Tgk * 128,
                            elem_size=64,
                            single_packet=False,
                            queue_num=q,
                            **kw,
                        )
                    if use_prep:
                        for q in sorted(used_q):
                            dma_count[q] += 1
                            nc.gpsimd.trigger_dma(count=None, queue_num=q)
                        for q in sorted(used_q):
                            nc.vector.wait_ge(dsems[q], 16 * dma_count[q])

                    if L == 1:
                        posl = mpool.tile([128, WG * G], dt.float32, tag="poh")
                        w0 = grp["windows"][0][0]
                        nwin = len(grp["windows"])
                        nc.sync.dma_start(
                            posl[:, : nwin * G],
                            pooloh[:, w0 * G : (w0 + nwin) * G],
                        )

                    for w, runs in grp["windows"]:
                        live = [(k, rel, Swk) for k, rel, Swk in runs if Swk > 0]
                        if len(live) == 0:
                            aggs = apool.tile([128, 64], dt.float32, tag="aggs")
                            nc.vector.memset(aggs[:], 0.0)
                        else:
                            if len(live) == 1:
                                k, rel, Swk = live[0]
                                agg_in = apool.tile([128, 64], dt.float32, tag="part")
                                nc.vector.tensor_reduce(
                                    agg_in[:],
                                    slab[:, rel * 64 : (rel + Swk) * 64].rearrange(
                                        "p (s d) -> p d s", d=64
                                    ),
                                    Ax.X,
                                    Alu.add,
                                )
                            else:
                                parts = ppool.tile(
                                    [128, len(live) * 64], dt.float32, tag="part"
                                )
                                for j, (k, rel, Swk) in enumerate(live):
                                    nc.vector.tensor_reduce(
                                        parts[:, j * 64 : (j + 1) * 64],
                                        slab[
                                            :, rel * 64 : (rel + Swk) * 64
                                        ].rearrange("p (s d) -> p d s", d=64),
                                        Ax.X,
                                        Alu.add,
                                    )
                                agg_in = apool.tile([128, 64], dt.float32, tag="agg")
                                nc.vector.tensor_reduce(
                                    agg_in[:],
                                    parts[:].rearrange("p (k d) -> p d k", d=64),
                                    Ax.X,
                                    Alu.add,
                                )
                            aggs = apool.tile([128, 64], dt.float32, tag="aggs")
                            nc.vector.tensor_scalar(
                                aggs[:], agg_in[:], invd_t[:, L * NW + w : L * NW + w + 1], None, Alu.mult
                            )
                        t1p = pspool.tile([64, 128], dt.float32, tag="t1")
                        nc.tensor.transpose(t1p[:], aggs[:], ident)
                        aggT = apool.tile([64, 128], dt.float32, tag="aggT")
                        nc.scalar.copy(aggT[:], t1p[:])
                        zps = pspool.tile([64, 128], dt.float32, tag="z")
                        nc.tensor.matmul(zps[:], lhsT=Wl, rhs=aggT[:], start=True, stop=True)
                        hT = apool.tile([64, 128], dt.float32, tag="hT")
                        nc.scalar.activation(hT[:], zps[:], Act.Relu, bias=bl)
                        t2p = pspool.tile([128, 64], dt.float32, tag="t2")
                        nc.tensor.transpose(t2p[:], hT[:], ident[:64, :64])
                        hwv = apool.tile([128, 64], dt.float32, tag="hw")
                        nc.scalar.copy(hwv[:], t2p[:])
                        if L == 0:
                            rows = min(128, S_ - w * 128)
                            nc.sync.dma_start(
                                h1_shard[w * 128 : w * 128 + rows, :], hwv[:rows, :]
                            )
                            if dbg_h1:
                                nc.sync.dma_start(
                                    h1dbg[w * 128 : w * 128 + rows, :], hwv[:rows, :]
                                )
                        else:
                            wrel = w - grp["windows"][0][0]
                            nc.tensor.matmul(
                                pool_ps[:],
                                lhsT=hwv[:],
                                rhs=posl[:, wrel * G : (wrel + 1) * G],
                                start=(w == 0),
                                stop=(w == NW - 1),
                            )

                    if L == 0:
                        for q in range(4):
                            if gi == [6, 12, 18, 24][q]:
                                nc.gpsimd.collective_compute(
                                    "AllGather",
                                    Alu.bypass,
                                    replica_groups=[list(range(NCORES))],
                                    ins=[h1_shard[q * QW : (q + 1) * QW, :]],
                                    outs=[h1g[q][0 : 8 * QW, :]],
                                )

            pooled_sb = apool.tile([64, G], dt.float32, tag="aggT")
            nc.scalar.copy(pooled_sb[:], pool_ps[:])
            nc.sync.dma_start(pooled_in[:], pooled_sb[:])
            nc.gpsimd.collective_compute(
                "AllReduce",
                Alu.add,
                replica_groups=[list(range(NCORES))],
                ins=[pooled_in[:]],
                outs=[pooled_out[:]],
            )
            pooledT = apool.tile([64, G], dt.float32, tag="hT")
            nc.sync.dma_start(pooledT[:], pooled_out[:])

            # head: logits = pooledT.T @ Wc + ones (x) bc
            ones_row = head_t[:, 0:128]
            bc_row = head_t[:, 128 : 128 + C]
            lps = pspool.tile([G, C], dt.float32, tag="z")
            nc.tensor.matmul(lps[:], lhsT=pooledT[:], rhs=wc_t[:], start=True, stop=False)
            nc.tensor.matmul(lps[:], lhsT=ones_row, rhs=bc_row, start=False, stop=True)
            lsb = apool.tile([G, C], dt.float32, tag="hw")
            nc.scalar.copy(lsb[:], lps[:])
            nc.sync.dma_start(logits[:], lsb[:])

    nc.finalize()
    return nc


def _run_timed(nc, in_maps, iters=1):
    """Mirror bass2jax.run_bass_via_pjrt's multi-core path, but keep inputs on
    device and execute `iters` times, timing each execution. Returns
    (results, times_s)."""
    import time

    import jax
    import numpy as _np
    from jax.experimental.shard_map import shard_map
    from jax.sharding import Mesh, NamedSharding, PartitionSpec

    import concourse.mybir as mybir
    from concourse import bass2jax

    bass2jax.install_neuronx_cc_hook()
    n_cores = len(in_maps)
    partition_name = nc.partition_id_tensor.name if nc.partition_id_tensor else None

    in_names, out_names, out_avals, zero_outs = [], [], [], []
    for alloc in nc.m.functions[0].allocations:
        if not isinstance(alloc, mybir.MemoryLocationSet):
            continue
        name = alloc.memorylocations[0].name
        if alloc.kind == "ExternalInput":
            if name != partition_name:
                in_names.append(name)
        elif alloc.kind == "ExternalOutput":
            out_names.append(name)
            shape = tuple(alloc.tensor_shape)
            dtype = mybir.dt.np(alloc.dtype)
            out_avals.append(jax.core.ShapedArray(shape, dtype))
            zero_outs.append(_np.zeros(shape, dtype))
    n_params = len(in_names)
    n_outs = len(out_avals)
    all_in_names = list(in_names) + out_names
    if partition_name is not None:
        all_in_names.append(partition_name)
    donate = tuple(range(n_params, n_params + n_outs))

    def _body(*args):
        operands = list(args)
        if partition_name is not None:
            operands.append(bass2jax.partition_id_tensor())
        outs = bass2jax._bass_exec_p.bind(
            *operands,
            out_avals=tuple(out_avals),
            in_names=tuple(all_in_names),
            out_names=tuple(out_names),
            lowering_input_output_aliases=(),
            sim_require_finite=True,
            sim_require_nnan=True,
            nc=nc,
        )
        return tuple(outs)

    devices = jax.devices()[:n_cores]
    mesh = Mesh(np.asarray(devices), ("core",))
    in_specs = (PartitionSpec("core"),) * (n_params + n_outs)
    out_specs = (PartitionSpec("core"),) * n_outs
    sharded = jax.jit(
        shard_map(_body, mesh=mesh, in_specs=in_specs, out_specs=out_specs, check_rep=False),
        donate_argnums=donate,
        keep_unused=True,
    )
    sh = NamedSharding(mesh, PartitionSpec("core"))
    concat_in = [
        jax.device_put(
            _np.concatenate([_np.asarray(in_maps[c][nm]) for c in range(n_cores)], axis=0),
            sh,
        )
        for nm in in_names
    ]
    times = []
    out_arrs = None
    for _ in range(max(1, iters)):
        concat_zeros = [
            jax.device_put(_np.zeros((n_cores * z.shape[0], *z.shape[1:]), z.dtype), sh)
            for z in zero_outs
        ]
        jax.block_until_ready(concat_zeros)
        t0 = time.perf_counter()
        out_arrs = sharded(*concat_in, *concat_zeros)
        jax.block_until_ready(out_arrs)
        times.append(time.perf_counter() - t0)
    B = int(os.environ.get("GNN_PIPE", "0"))
    if B > 1:
        zsets = [
            [
                jax.device_put(
                    _np.zeros((n_cores * z.shape[0], *z.shape[1:]), z.dtype), sh
                )
                for z in zero_outs
            ]
            for _ in range(B)
        ]
        jax.block_until_ready(zsets)
        t0 = time.perf_counter()
        outs = [sharded(*concat_in, *zs) for zs in zsets]
        jax.block_until_ready(outs)
        tot = time.perf_counter() - t0
        single = min(times)
        print(
            f"pipelined x{B}: total={tot * 1e3:.2f}ms  single={single * 1e3:.2f}ms  "
            f"marginal={(tot - single) / (B - 1) * 1e6:.0f}us"
        )
        times.append(max((tot - single) / (B - 1), 1e-9))
    results = [
        {
            nm: _np.asarray(out_arrs[i]).reshape(n_cores, *out_avals[i].shape)[c]
            for i, nm in enumerate(out_names)
        }
        for c in range(n_cores)
    ]
    return results, times


def build(inputs):
    """Build (nc, in_maps) for the current inputs. Shared by kernel() and
    the profiling harness."""
    tokens = np.asarray(inputs["tokens"]).astype(np.int64)
    edge_src = np.asarray(inputs["edge_src"]).astype(np.int64)
    edge_dst = np.asarray(inputs["edge_dst"]).astype(np.int64)
    graph_ids = np.asarray(inputs["graph_ids"]).astype(np.int64)
    emb = np.asarray(inputs["emb_table"], f32)
    W1 = np.asarray(inputs["W1"], f32)
    b1 = np.asarray(inputs["b1"], f32)
    W2 = np.asarray(inputs["W2"], f32)
    b2 = np.asarray(inputs["b2"], f32)
    Wc = np.asarray(inputs["Wc"], f32)
    bc = np.asarray(inputs["bc"], f32)

    s1, s2, idx1, idx2, invd, pooloh, cnt, p1s = _prep(tokens, edge_src, edge_dst, graph_ids)
    if os.environ.get("GNN_VERBOSE", "0") == "1":
        print(f"T1={s1['T']} tiles ({s1['T'] * 128} rows), "
              f"T2={s2['T']} tiles ({s2['T'] * 128} rows)")

    embg = np.zeros((NCH1 * CHROWS1, D), f32)
    for ch in range(NCH1):
        embg[ch * CHROWS1 : ch * CHROWS1 + CW1] = emb[ch * CW1 : (ch + 1) * CW1]

    consts = np.eye(128, dtype=f32)
    wts = np.concatenate([W1, W2, b1[:, None], b2[:, None]], axis=1)
    head_arr = np.zeros((1, 128 + C), f32)
    head_arr[0, :128] = 1.0
    head_arr[0, 128 : 128 + C] = bc

    nc = _build(s1, s2)

    in_maps = []
    for c in range(NCORES):
        in_maps.append(
            {
                "embg": embg,
                "consts": consts,
                "wts": wts,
                "head": head_arr,
                "wc": Wc,
                "invd": invd[c],
                "pooloh": pooloh[c],
                "l1idx": idx1[c],
                "l2idx": idx2[c],
            }
        )
    return nc, in_maps


def kernel(**inputs):
    global last_result

    nc, in_maps = build(inputs)

    iters = int(os.environ.get("GNN_BENCH", "1"))
    results, times = _run_timed(nc, in_maps, iters=iters)
    last_result = {"times": times, "results": results}
    if iters > 1:
        print(f"exec times (s): {[f'{t * 1e3:.2f}ms' for t in times]}")
        print(f"best exec: {min(times) * 1e6:.0f} us")
    return np.asarray(results[0]["logits"], f32)
